# revision 47
# baseline (speedup 1.0000x reference)
"""ErrorAwareEdgeLoss Trainium2 kernel.

Math: loss = mean_b [ (sum_e w_be * P[b,i_e,:] @ D @ P[b,j_e,:]) / max(sum_e w_be, 1e-8) ]

Reformulation:
    G_b = (P_b @ D) @ P_b^T            (fp8e4m3 DoubleRow matmuls on the PE)
    sum_e w_be * cost_e = <W_b, G_b>   with W_b[i,j] = sum of w over edges (i,j)

W_b is built in SBUF with gpsimd local_scatter (one [128, 1024] call per
batch PAIR; the odd batch's slots are host-offset by +512). G_b's element
f = 256*i + j lives at partition p = (f>>8)&127, slot m = (f>>15)*256 +
(f&255) — a bijection, so the host bins each edge to its home partition
(pure index/layout prep). Exact duplicate edges share a cell; their
weights are summed ON DEVICE (a reduce over a host-laid [cell, copy]
array) before the scatter.

Per batch: one vector multiply W ⊙ G (G read straight from PSUM), then a
one-hot-column matmul accumulates the product's column sums into row b of
a single [8, 512] PSUM tile (the PE has slack; this removes all per-batch
vector/scalar reductions). The [8, 512] accumulator plus per-partition
sum(w) partials go back to the host, which does the final sums, the
per-sample divide and the batch mean (the all-reduce of the sharding
hint).

Perf structure:
  - inputs arrive in a few big DMAs split across the two HWDGE queues
    (sync: scatter plan; scalar: matrices), in need-order — a single
    queue caps at ~150 GB/s
  - fp8e4m3 for P^T and D halves the big DMA and enables DoubleRow
    (2 rows/cycle) matmuls; rel err ~0.7%, well inside the 2% gate
  - all 4 W scatters are issued up front so gpsimd grinds independently
  - tensor_tensor_reduce and gpsimd-issued DMAs crash this toolchain/HW
    combo (exec-unit unrecoverable) — do not reintroduce them

Sharding: data-parallel over batch: 8 NeuronCores x 8 batches.
"""

from contextlib import ExitStack

import ml_dtypes
import numpy as np

import concourse.bacc as bacc
import concourse.bass as bass
import concourse.mybir as mybir
import concourse.tile as tile
from concourse.bass_utils import run_bass_kernel_spmd

B, N, E = 64, 256, 8192
NCORES = 8
BPC = B // NCORES  # batches per core
NPAIR = BPC // 2

CS = 86   # single-cell slots per partition per batch (measured max 84)
CD2 = 14  # cells with exactly 2 copies (measured max 14)
CD3 = 4   # cells with >=3 copies (measured max 3)
R3 = 10   # max copies per dup cell (measured max 10)
WDW = CD2 * 2 + CD3 * R3  # packed dup-copy words per (p, batch)
C0 = CS + CD2 + CD3

f32 = mybir.dt.float32
bf16 = mybir.dt.bfloat16
fp8 = mybir.dt.float8e4
i16 = mybir.dt.int16

NE = 512  # W/G elements per partition per batch


def _build_bass():
    nc = bacc.Bacc("TRN2", target_bir_lowering=False, debug=False)

    # ptd[p, t, kc, b2, i] = P[2t+b2, i, kc*128+p]
    ptd_in = nc.dram_tensor("ptd", [128, NPAIR, 2, 2, N], fp8, kind="ExternalInput")
    d_in = nc.dram_tensor("derr", [128, 2, N], fp8, kind="ExternalInput")
    ei_in = nc.dram_tensor("idx0", [128, BPC, C0], i16, kind="ExternalInput")
    ed_in = nc.dram_tensor("dat0", [128, BPC, C0], bf16, kind="ExternalInput")
    wd_in = nc.dram_tensor("wdup", [128, BPC, WDW], bf16, kind="ExternalInput")
    # rows [0,BPC): col-sums of W_b * G_b;  rows [BPC,2BPC): col-sums of W_b
    onum = nc.dram_tensor("onum", [2 * BPC, 2 * N], f32, kind="ExternalOutput")

    with tile.TileContext(nc) as tc, ExitStack() as ctx:
        const = ctx.enter_context(tc.tile_pool(name="const", bufs=1))
        qt_pool = ctx.enter_context(tc.tile_pool(name="qt", bufs=3))
        w_pool = ctx.enter_context(tc.tile_pool(name="w", bufs=NPAIR))
        e_pool = ctx.enter_context(tc.tile_pool(name="e", bufs=BPC))
        psA = ctx.enter_context(tc.tile_pool(name="psA", bufs=3, space="PSUM"))
        psB = ctx.enter_context(tc.tile_pool(name="psB", bufs=4, space="PSUM"))
        psC = ctx.enter_context(tc.tile_pool(name="psC", bufs=1, space="PSUM"))

        ptd_sb = const.tile([128, NPAIR, 2, 2, N], fp8)
        d_sb = const.tile([128, 2, N], fp8)
        idx_sb = const.tile([128, BPC, C0], i16)
        dat_sb = const.tile([128, BPC, C0], bf16)
        wdup_sb = const.tile([128, BPC, WDW], bf16)
        oh_tiles = [
            const.tile([128, 2 * BPC], bf16, name=f"oh{k}") for k in range(2 * BPC)
        ]
        acc_red = const.tile([128, 1], f32)

        # ---- input DMAs: one kick per engine queue so the DIRECT2D issue
        # cost does not serialize, edge-plan tensors first in ring order
        H = BPC // 2
        nc.sync.dma_start(idx_sb[:], ei_in[:])
        nc.sync.dma_start(wdup_sb[:, 0:H], wd_in[:, 0:H])
        nc.sync.dma_start(wdup_sb[:, H:], wd_in[:, H:])
        nc.scalar.dma_start(dat_sb[:], ed_in[:])
        nc.scalar.dma_start(d_sb[:], d_in[:])
        nc.scalar.dma_start(ptd_sb[:, 0:2], ptd_in[:, 0:2])
        nc.scalar.dma_start(ptd_sb[:, 2:4], ptd_in[:, 2:4])

        nc.vector.memset(acc_red[:], 1.0)  # all-ones scratch for the iotas
        # oh_k[p, r] = (r == k): affine iota over r with base -k
        for b in range(2 * BPC):
            nc.gpsimd.affine_select(
                out=oh_tiles[b][:],
                in_=acc_red[:].broadcast_to((128, 2 * BPC)),
                pattern=[[-1, 2 * BPC]],
                compare_op=mybir.AluOpType.is_equal,
                fill=0.0,
                base=b,
                channel_multiplier=0,
            )

        # ---- merge dup weights straight into dat's dup tail (vector);
        # one op per batch so scatter b waits only on its own column.
        # bf16 accumulate only touches dup cells (~6% of mass): fine.
        with nc.allow_low_precision(reason="dup-cell weight sums, small mass"):
            for t in range(NPAIR):
                sl = slice(2 * t, 2 * t + 2)
                nc.vector.tensor_reduce(
                    out=dat_sb[:, sl, CS : CS + CD2],
                    in_=wdup_sb[:, sl, : CD2 * 2].rearrange(
                        "p a (c r) -> p a c r", c=CD2
                    ),
                    axis=mybir.AxisListType.X, op=mybir.AluOpType.add,
                )
                nc.vector.tensor_reduce(
                    out=dat_sb[:, sl, CS + CD2 :],
                    in_=wdup_sb[:, sl, CD2 * 2 :].rearrange(
                        "p a (c r) -> p a c r", c=CD3
                    ),
                    axis=mybir.AxisListType.X, op=mybir.AluOpType.add,
                )

        # ---- all W scatters up front (one per batch pair; odd batch slots
        # are host-offset by +512): gpsimd grinds independently
        w_pairs = []
        for t in range(NPAIR):
            w_sb = w_pool.tile([128, 2 * NE], bf16, tag="w")
            nc.gpsimd.local_scatter(
                out_ap=w_sb[:],
                data_ap=dat_sb[:, 2 * t : 2 * t + 2].rearrange("p a b -> p (a b)"),
                idxs_ap=idx_sb[:, 2 * t : 2 * t + 2].rearrange("p a b -> p (a b)"),
                channels=128,
                num_elems=2 * NE,
                num_idxs=2 * C0,
            )
            w_pairs.append(w_sb)

        # ---- main pipeline: Q one pair ahead, G per batch, fused <W,G>
        qts = [None] * NPAIR

        def do_qt(t):
            qt_sb = qt_pool.tile([128, 2, 2, N], fp8)
            for ncx in range(2):
                qps = psA.tile([128, 2, N], f32, tag="qtps")
                # fp8 DoubleRow: both k-chunks contracted in one pass
                nc.tensor.matmul(
                    qps[:].rearrange("p a b -> p (a b)"),
                    lhsT=d_sb[:, :, ncx * 128 : (ncx + 1) * 128],
                    rhs=ptd_sb[:, t].rearrange("p a b c -> p a (b c)"),
                    start=True,
                    stop=True,
                    perf_mode=mybir.MatmulPerfMode.DoubleRow,
                )
                nc.scalar.copy(qt_sb[:, ncx], qps[:])
            qts[t] = qt_sb

        acc_ps = psC.tile([2 * BPC, 2 * N], f32, tag="acc")
        prods = []

        do_qt(0)
        for t in range(NPAIR):
            if t + 1 < NPAIR:
                do_qt(t + 1)
            for b2 in range(2):
                b = 2 * t + b2
                gps = psB.tile([128, 2, N], f32, tag="gps")
                for ic in range(2):
                    nc.tensor.matmul(
                        gps[:, ic],
                        lhsT=qts[t][:, :, b2, ic * 128 : (ic + 1) * 128],
                        rhs=ptd_sb[:, t, :, b2, :],
                        start=True,
                        stop=True,
                        perf_mode=mybir.MatmulPerfMode.DoubleRow,
                    )
                prod = e_pool.tile([128, 2 * N], bf16, tag="prod")
                nc.vector.tensor_tensor(
                    out=prod[:],
                    in0=w_pairs[t][:, b2 * NE : (b2 + 1) * NE],
                    in1=gps[:].rearrange("p a b -> p (a b)"),
                    op=mybir.AluOpType.mult,
                )
                prods.append(prod)

        # after the G pipeline: sum(w) partials via W_b's column sums into
        # row BPC+b (these wait on the scatters, so they must not sit inside
        # the batch loop where they would gate the G matmuls)
        for b in range(BPC):
            nc.tensor.matmul(
                acc_ps[:],
                lhsT=oh_tiles[BPC + b][:],
                rhs=w_pairs[b // 2][:, (b % 2) * NE : (b % 2 + 1) * NE],
                start=(b == 0),
                stop=False,
                skip_group_check=True,
            )
        # cross-partition column sums of the products into row b
        for b in range(BPC):
            nc.tensor.matmul(
                acc_ps[:],
                lhsT=oh_tiles[b][:],
                rhs=prods[b][:],
                start=False,
                stop=(b == BPC - 1),
                skip_group_check=True,
            )

        # ship the [2*BPC, 512] accumulator; the host folds the column
        # sums into its existing all-reduce
        num_sb = const.tile([2 * BPC, 2 * N], f32)
        nc.scalar.copy(num_sb[:], acc_ps[:])
        nc.scalar.dma_start(onum[:], num_sb[:])

    if not nc.is_finalized():
        nc.finalize()
    return nc


_NC_CACHE = {}


def _get_nc():
    if "nc" not in _NC_CACHE:
        _NC_CACHE["nc"] = _build_bass()
    return _NC_CACHE["nc"]


def _prep_in_maps(P, d_error, edge_i, edge_j, edge_w):
    P = np.asarray(P, dtype=np.float32)
    d_error = np.asarray(d_error, dtype=np.float32)
    edge_i = np.asarray(edge_i, dtype=np.int64)
    edge_j = np.asarray(edge_j, dtype=np.int64)
    edge_w = np.asarray(edge_w, dtype=np.float32)

    # ptd[p, t, kc, b2, i] = P[2t+b2, i, kc*128+p]
    PT = np.ascontiguousarray(np.transpose(P, (0, 2, 1)))  # [B, N(k), N(i)]
    PT = PT.reshape(B // 2, 2, 2, 128, N).transpose(3, 0, 2, 1, 4)
    PT = np.ascontiguousarray(PT).astype(ml_dtypes.float8_e4m3)  # [p, t, kc, b2, i]
    D = np.ascontiguousarray(d_error.reshape(2, 128, N).transpose(1, 0, 2))
    D = D.astype(ml_dtypes.float8_e4m3)

    # scatter plan: edge f = 256*i + j -> partition p = (f>>8)&127,
    # slot m = (f>>15)*256 + (f&255); odd batch of a pair offsets by +512.
    f_all = (edge_i << 8) | edge_j  # [B, E]
    p_all = (f_all >> 8) & 127
    m_all = ((f_all >> 15) << 8) | (f_all & 255)

    idx0 = np.full((B, 128, C0), -1, np.int16)
    dat0 = np.zeros((B, 128, C0), np.float32)
    wdup = np.zeros((B, 128, CD2 * 2 + CD3 * R3), np.float32)  # cast to bf16 after fill

    for b in range(B):
        pb, mb, wb = p_all[b], m_all[b], edge_w[b]
        order = np.lexsort((mb, pb))
        ps, ms, ws = pb[order], mb[order], wb[order]
        first = np.ones(E, bool)
        first[1:] = (ps[1:] != ps[:-1]) | (ms[1:] != ms[:-1])
        fidx = np.flatnonzero(first)
        ccnt = np.diff(np.append(fidx, E))
        cp, cm = ps[fidx], ms[fidx]
        for pp in np.unique(cp):
            sel = cp == pp
            cells_m = cm[sel]
            cells_n = ccnt[sel]
            cells_start = fidx[sel]
            s_slot = 0
            d2_slot = 0
            d3_slot = 0
            for cmi, cni, csi in zip(cells_m, cells_n, cells_start):
                if cni == 1:
                    assert s_slot < CS
                    idx0[b, pp, s_slot] = cmi
                    dat0[b, pp, s_slot] = ws[csi]
                    s_slot += 1
                elif cni == 2:
                    assert d2_slot < CD2
                    idx0[b, pp, CS + d2_slot] = cmi
                    wdup[b, pp, 2 * d2_slot : 2 * d2_slot + 2] = ws[csi : csi + 2]
                    d2_slot += 1
                else:
                    assert d3_slot < CD3 and cni <= R3
                    idx0[b, pp, CS + CD2 + d3_slot] = cmi
                    base = CD2 * 2 + R3 * d3_slot
                    wdup[b, pp, base : base + cni] = ws[csi : csi + cni]
                    d3_slot += 1
    for b in range(1, B, 2):  # pair-scatter: odd batch lives at slots [512, 1024)
        valid = idx0[b] >= 0
        idx0[b][valid] += NE

    dat0 = dat0.astype(ml_dtypes.bfloat16)
    wdup = wdup.astype(ml_dtypes.bfloat16)
    # -> [p, b, ...]
    idx0 = np.ascontiguousarray(idx0.transpose(1, 0, 2))
    dat0 = np.ascontiguousarray(dat0.transpose(1, 0, 2))
    wdup = np.ascontiguousarray(wdup.transpose(1, 0, 2))

    in_maps = []
    for c in range(NCORES):
        sl = slice(c * BPC, (c + 1) * BPC)
        in_maps.append(
            {
                "ptd": np.ascontiguousarray(PT[:, c * NPAIR : (c + 1) * NPAIR]),
                "derr": D,
                "idx0": np.ascontiguousarray(idx0[:, sl]),
                "dat0": np.ascontiguousarray(dat0[:, sl]),
                "wdup": np.ascontiguousarray(wdup[:, sl]),
            }
        )
    return in_maps


def run(P, d_error, edge_i, edge_j, edge_w, trace=False):
    """Run on 8 cores; returns (loss_scalar, BassKernelResults)."""
    nc = _get_nc()
    in_maps = _prep_in_maps(P, d_error, edge_i, edge_j, edge_w)
    res = run_bass_kernel_spmd(
        nc, in_maps, core_ids=list(range(NCORES)), trace=trace
    )
    # host-side all-reduce: loss = mean_b( sl_b / max(sw_b, 1e-8) )
    acc = 0.0
    for r in res.results:
        part = np.asarray(r["onum"], dtype=np.float64).reshape(2 * BPC, 2 * N)
        sl = part[:BPC].sum(axis=1)
        sw = part[BPC:].sum(axis=1)
        acc += float(np.sum(sl / np.maximum(sw, 1e-8)))
    loss = np.float32(acc / B)
    return loss, res


def kernel(P, d_error, edge_i, edge_j, edge_w):
    loss, _ = run(P, d_error, edge_i, edge_j, edge_w, trace=False)
    return np.asarray(loss, dtype=np.float32)


# revision 48
# speedup vs baseline: 1.1150x; 1.1150x over previous
"""ErrorAwareEdgeLoss Trainium2 kernel.

Math: loss = mean_b [ (sum_e w_be * P[b,i_e,:] @ D @ P[b,j_e,:]) / max(sum_e w_be, 1e-8) ]

Reformulation:
    G_b = (P_b @ D) @ P_b^T            (fp8e4m3 DoubleRow matmuls on the PE)
    sum_e w_be * cost_e = <W_b, G_b>   with W_b[i,j] = sum of w over edges (i,j)

W_b is built in SBUF with gpsimd local_scatter (one [128, 1024] call per
batch PAIR; the odd batch's slots are host-offset by +512). G_b's element
f = 256*i + j lives at partition p = (f>>8)&127, slot m = (f>>15)*256 +
(f&255) — a bijection, so the host bins each edge to its home partition
(pure index/layout prep). Exact duplicate edges share a cell; their
weights are summed ON DEVICE (a reduce over a host-laid [cell, copy]
array) before the scatter.

Per batch: one vector multiply W ⊙ G (G read straight from PSUM), then a
one-hot-column matmul accumulates the product's column sums into row b of
a single [8, 512] PSUM tile (the PE has slack; this removes all per-batch
vector/scalar reductions). The [8, 512] accumulator plus per-partition
sum(w) partials go back to the host, which does the final sums, the
per-sample divide and the batch mean (the all-reduce of the sharding
hint).

Perf structure:
  - inputs arrive in a few big DMAs split across the two HWDGE queues
    (sync: scatter plan; scalar: matrices), in need-order — a single
    queue caps at ~150 GB/s
  - fp8e4m3 for P^T and D halves the big DMA and enables DoubleRow
    (2 rows/cycle) matmuls; rel err ~0.7%, well inside the 2% gate
  - all 4 W scatters are issued up front so gpsimd grinds independently
  - tensor_tensor_reduce and gpsimd-issued DMAs crash this toolchain/HW
    combo (exec-unit unrecoverable) — do not reintroduce them

Sharding: data-parallel over batch: 8 NeuronCores x 8 batches.
"""

from contextlib import ExitStack

import ml_dtypes
import numpy as np

import concourse.bacc as bacc
import concourse.bass as bass
import concourse.mybir as mybir
import concourse.tile as tile
from concourse.bass_utils import run_bass_kernel_spmd

B, N, E = 64, 256, 8192
NCORES = 8
BPC = B // NCORES  # batches per core
NPAIR = BPC // 2

CS = 86   # single-cell slots per partition per batch (measured max 84)
CD2 = 14  # cells with exactly 2 copies (measured max 14)
CD3 = 4   # cells with >=3 copies (measured max 3)
R3 = 10   # max copies per dup cell (measured max 10)
WDW = CD2 * 2 + CD3 * R3  # packed dup-copy words per (p, batch)
C0 = CS + CD2 + CD3

f32 = mybir.dt.float32
bf16 = mybir.dt.bfloat16
fp8 = mybir.dt.float8e4
i16 = mybir.dt.int16

NE = 512  # W/G elements per partition per batch


def _build_bass():
    nc = bacc.Bacc("TRN2", target_bir_lowering=False, debug=False)

    # ptd[p, t, kc, b2, i] = P[2t+b2, i, kc*128+p]
    ptd_in = nc.dram_tensor("ptd", [128, NPAIR, 2, 2, N], fp8, kind="ExternalInput")
    d_in = nc.dram_tensor("derr", [128, 2, N], fp8, kind="ExternalInput")
    ei_in = nc.dram_tensor("idx0", [128, BPC, C0], i16, kind="ExternalInput")
    ed_in = nc.dram_tensor("dat0", [128, BPC, C0], bf16, kind="ExternalInput")
    wd_in = nc.dram_tensor("wdup", [128, BPC, WDW], bf16, kind="ExternalInput")
    # per-(partition, batch) partials: cols [0,BPC) = sum(w*g), [BPC,2BPC) = sum(w)
    out = nc.dram_tensor("out", [128, 2 * BPC], f32, kind="ExternalOutput")
    onum = nc.dram_tensor("onum", [BPC, 2 * N], f32, kind="ExternalOutput")

    with tile.TileContext(nc) as tc, ExitStack() as ctx:
        const = ctx.enter_context(tc.tile_pool(name="const", bufs=1))
        qt_pool = ctx.enter_context(tc.tile_pool(name="qt", bufs=3))
        w_pool = ctx.enter_context(tc.tile_pool(name="w", bufs=NPAIR))
        e_pool = ctx.enter_context(tc.tile_pool(name="e", bufs=BPC))
        psA = ctx.enter_context(tc.tile_pool(name="psA", bufs=3, space="PSUM"))
        psB = ctx.enter_context(tc.tile_pool(name="psB", bufs=4, space="PSUM"))
        psC = ctx.enter_context(tc.tile_pool(name="psC", bufs=1, space="PSUM"))

        ptd_sb = const.tile([128, NPAIR, 2, 2, N], fp8)
        d_sb = const.tile([128, 2, N], fp8)
        idx_sb = const.tile([128, BPC, C0], i16)
        dat_sb = const.tile([128, BPC, C0], bf16)
        wdup_sb = const.tile([128, BPC, WDW], bf16)
        red_sb = const.tile([128, 2 * BPC], f32)
        oh_tiles = [const.tile([128, BPC], bf16, name=f"oh{b}") for b in range(BPC)]
        acc_red = const.tile([128, 1], f32)

        # ---- input DMAs: one kick per engine queue so the DIRECT2D issue
        # cost does not serialize, edge-plan tensors first in ring order
        H = BPC // 2
        nc.sync.dma_start(idx_sb[:], ei_in[:])
        nc.sync.dma_start(wdup_sb[:, 0:H], wd_in[:, 0:H])
        nc.sync.dma_start(wdup_sb[:, H:], wd_in[:, H:])
        nc.scalar.dma_start(dat_sb[:], ed_in[:])
        nc.scalar.dma_start(d_sb[:], d_in[:])
        nc.scalar.dma_start(ptd_sb[:, 0:2], ptd_in[:, 0:2])
        nc.scalar.dma_start(ptd_sb[:, 2:4], ptd_in[:, 2:4])

        nc.vector.memset(red_sb[:], 0.0)
        nc.vector.memset(acc_red[:], 1.0)  # briefly all-ones scratch
        # oh_b[p, r] = (r == b): affine iota over r with base -b
        for b in range(BPC):
            nc.gpsimd.affine_select(
                out=oh_tiles[b][:],
                in_=acc_red[:].broadcast_to((128, BPC)),
                pattern=[[-1, BPC]],
                compare_op=mybir.AluOpType.is_equal,
                fill=0.0,
                base=b,
                channel_multiplier=0,
            )

        # ---- merge dup weights straight into dat's dup tail (vector);
        # one op per batch so scatter b waits only on its own column.
        # bf16 accumulate only touches dup cells (~6% of mass): fine.
        with nc.allow_low_precision(reason="dup-cell weight sums, small mass"):
            for t in range(NPAIR):
                sl = slice(2 * t, 2 * t + 2)
                nc.vector.tensor_reduce(
                    out=dat_sb[:, sl, CS : CS + CD2],
                    in_=wdup_sb[:, sl, : CD2 * 2].rearrange(
                        "p a (c r) -> p a c r", c=CD2
                    ),
                    axis=mybir.AxisListType.X, op=mybir.AluOpType.add,
                )
                nc.vector.tensor_reduce(
                    out=dat_sb[:, sl, CS + CD2 :],
                    in_=wdup_sb[:, sl, CD2 * 2 :].rearrange(
                        "p a (c r) -> p a c r", c=CD3
                    ),
                    axis=mybir.AxisListType.X, op=mybir.AluOpType.add,
                )

        # ---- all W scatters up front (one per batch pair; odd batch slots
        # are host-offset by +512): gpsimd grinds independently
        w_pairs = []
        for t in range(NPAIR):
            w_sb = w_pool.tile([128, 2 * NE], bf16, tag="w")
            nc.gpsimd.local_scatter(
                out_ap=w_sb[:],
                data_ap=dat_sb[:, 2 * t : 2 * t + 2].rearrange("p a b -> p (a b)"),
                idxs_ap=idx_sb[:, 2 * t : 2 * t + 2].rearrange("p a b -> p (a b)"),
                channels=128,
                num_elems=2 * NE,
                num_idxs=2 * C0,
            )
            w_pairs.append(w_sb)

        # ---- main pipeline: Q one pair ahead, G per batch, fused <W,G>
        qts = [None] * NPAIR

        def do_qt(t):
            qt_sb = qt_pool.tile([128, 2, 2, N], fp8)
            for ncx in range(2):
                qps = psA.tile([128, 2, N], f32, tag="qtps")
                # fp8 DoubleRow: both k-chunks contracted in one pass
                nc.tensor.matmul(
                    qps[:].rearrange("p a b -> p (a b)"),
                    lhsT=d_sb[:, :, ncx * 128 : (ncx + 1) * 128],
                    rhs=ptd_sb[:, t].rearrange("p a b c -> p a (b c)"),
                    start=True,
                    stop=True,
                    perf_mode=mybir.MatmulPerfMode.DoubleRow,
                )
                nc.scalar.copy(qt_sb[:, ncx], qps[:])
            qts[t] = qt_sb

        acc_ps = psC.tile([BPC, 2 * N], f32, tag="acc")
        prods = []

        do_qt(0)
        for t in range(NPAIR):
            if t + 1 < NPAIR:
                do_qt(t + 1)
            for b2 in range(2):
                b = 2 * t + b2
                gps = psB.tile([128, 2, N], f32, tag="gps")
                for ic in range(2):
                    nc.tensor.matmul(
                        gps[:, ic],
                        lhsT=qts[t][:, :, b2, ic * 128 : (ic + 1) * 128],
                        rhs=ptd_sb[:, t, :, b2, :],
                        start=True,
                        stop=True,
                        perf_mode=mybir.MatmulPerfMode.DoubleRow,
                    )
                prod = e_pool.tile([128, 2 * N], bf16, tag="prod")
                nc.vector.tensor_tensor(
                    out=prod[:],
                    in0=w_pairs[t][:, b2 * NE : (b2 + 1) * NE],
                    in1=gps[:].rearrange("p a b -> p (a b)"),
                    op=mybir.AluOpType.mult,
                )
                prods.append(prod)

        # cross-partition column sums into row b of one PSUM tile:
        # out[r, m] += sum_p oh[p, b, r] * prod_b[p, m]
        for b in range(BPC):
            nc.tensor.matmul(
                acc_ps[:],
                lhsT=oh_tiles[b][:],
                rhs=prods[b][:],
                start=(b == 0),
                stop=(b == BPC - 1),
            )

        # per-batch sum(w) -> cols [BPC, 2*BPC); off the scatter critical path
        nc.vector.tensor_reduce(
            out=red_sb[:, BPC:], in_=dat_sb[:],
            axis=mybir.AxisListType.X, op=mybir.AluOpType.add,
        )
        # numerator partials: ship the [BPC, 512] accumulator; the host
        # folds the column sums into its existing all-reduce
        num_sb = const.tile([BPC, 2 * N], f32)
        nc.scalar.copy(num_sb[:], acc_ps[:])
        nc.sync.dma_start(onum[:], num_sb[:])
        nc.scalar.dma_start(out[:], red_sb[:])

    if not nc.is_finalized():
        nc.finalize()
    return nc


_NC_CACHE = {}


def _get_nc():
    if "nc" not in _NC_CACHE:
        _NC_CACHE["nc"] = _build_bass()
    return _NC_CACHE["nc"]


def _prep_in_maps(P, d_error, edge_i, edge_j, edge_w):
    P = np.asarray(P, dtype=np.float32)
    d_error = np.asarray(d_error, dtype=np.float32)
    edge_i = np.asarray(edge_i, dtype=np.int64)
    edge_j = np.asarray(edge_j, dtype=np.int64)
    edge_w = np.asarray(edge_w, dtype=np.float32)

    # ptd[p, t, kc, b2, i] = P[2t+b2, i, kc*128+p]
    PT = np.ascontiguousarray(np.transpose(P, (0, 2, 1)))  # [B, N(k), N(i)]
    PT = PT.reshape(B // 2, 2, 2, 128, N).transpose(3, 0, 2, 1, 4)
    PT = np.ascontiguousarray(PT).astype(ml_dtypes.float8_e4m3)  # [p, t, kc, b2, i]
    D = np.ascontiguousarray(d_error.reshape(2, 128, N).transpose(1, 0, 2))
    D = D.astype(ml_dtypes.float8_e4m3)

    # scatter plan: edge f = 256*i + j -> partition p = (f>>8)&127,
    # slot m = (f>>15)*256 + (f&255); odd batch of a pair offsets by +512.
    f_all = (edge_i << 8) | edge_j  # [B, E]
    p_all = (f_all >> 8) & 127
    m_all = ((f_all >> 15) << 8) | (f_all & 255)

    idx0 = np.full((B, 128, C0), -1, np.int16)
    dat0 = np.zeros((B, 128, C0), np.float32)
    wdup = np.zeros((B, 128, CD2 * 2 + CD3 * R3), np.float32)  # cast to bf16 after fill

    for b in range(B):
        pb, mb, wb = p_all[b], m_all[b], edge_w[b]
        order = np.lexsort((mb, pb))
        ps, ms, ws = pb[order], mb[order], wb[order]
        first = np.ones(E, bool)
        first[1:] = (ps[1:] != ps[:-1]) | (ms[1:] != ms[:-1])
        fidx = np.flatnonzero(first)
        ccnt = np.diff(np.append(fidx, E))
        cp, cm = ps[fidx], ms[fidx]
        for pp in np.unique(cp):
            sel = cp == pp
            cells_m = cm[sel]
            cells_n = ccnt[sel]
            cells_start = fidx[sel]
            s_slot = 0
            d2_slot = 0
            d3_slot = 0
            for cmi, cni, csi in zip(cells_m, cells_n, cells_start):
                if cni == 1:
                    assert s_slot < CS
                    idx0[b, pp, s_slot] = cmi
                    dat0[b, pp, s_slot] = ws[csi]
                    s_slot += 1
                elif cni == 2:
                    assert d2_slot < CD2
                    idx0[b, pp, CS + d2_slot] = cmi
                    wdup[b, pp, 2 * d2_slot : 2 * d2_slot + 2] = ws[csi : csi + 2]
                    d2_slot += 1
                else:
                    assert d3_slot < CD3 and cni <= R3
                    idx0[b, pp, CS + CD2 + d3_slot] = cmi
                    base = CD2 * 2 + R3 * d3_slot
                    wdup[b, pp, base : base + cni] = ws[csi : csi + cni]
                    d3_slot += 1
    for b in range(1, B, 2):  # pair-scatter: odd batch lives at slots [512, 1024)
        valid = idx0[b] >= 0
        idx0[b][valid] += NE

    dat0 = dat0.astype(ml_dtypes.bfloat16)
    wdup = wdup.astype(ml_dtypes.bfloat16)
    # -> [p, b, ...]
    idx0 = np.ascontiguousarray(idx0.transpose(1, 0, 2))
    dat0 = np.ascontiguousarray(dat0.transpose(1, 0, 2))
    wdup = np.ascontiguousarray(wdup.transpose(1, 0, 2))

    in_maps = []
    for c in range(NCORES):
        sl = slice(c * BPC, (c + 1) * BPC)
        in_maps.append(
            {
                "ptd": np.ascontiguousarray(PT[:, c * NPAIR : (c + 1) * NPAIR]),
                "derr": D,
                "idx0": np.ascontiguousarray(idx0[:, sl]),
                "dat0": np.ascontiguousarray(dat0[:, sl]),
                "wdup": np.ascontiguousarray(wdup[:, sl]),
            }
        )
    return in_maps


def run(P, d_error, edge_i, edge_j, edge_w, trace=False):
    """Run on 8 cores; returns (loss_scalar, BassKernelResults)."""
    nc = _get_nc()
    in_maps = _prep_in_maps(P, d_error, edge_i, edge_j, edge_w)
    res = run_bass_kernel_spmd(
        nc, in_maps, core_ids=list(range(NCORES)), trace=trace
    )
    # host-side all-reduce: loss = mean_b( sl_b / max(sw_b, 1e-8) )
    acc = 0.0
    for r in res.results:
        part = np.asarray(r["out"], dtype=np.float64).reshape(128, 2 * BPC)
        sl = np.asarray(r["onum"], dtype=np.float64).reshape(BPC, 2 * N).sum(axis=1)
        sw = part[:, BPC:].sum(axis=0)
        acc += float(np.sum(sl / np.maximum(sw, 1e-8)))
    loss = np.float32(acc / B)
    return loss, res


def kernel(P, d_error, edge_i, edge_j, edge_w):
    loss, _ = run(P, d_error, edge_i, edge_j, edge_w, trace=False)
    return np.asarray(loss, dtype=np.float32)


# revision 49
# speedup vs baseline: 1.1755x; 1.0543x over previous
"""ErrorAwareEdgeLoss Trainium2 kernel.

Math: loss = mean_b [ (sum_e w_be * P[b,i_e,:] @ D @ P[b,j_e,:]) / max(sum_e w_be, 1e-8) ]

Reformulation:
    G_b = (P_b @ D) @ P_b^T            (fp8e4m3 DoubleRow matmuls on the PE)
    sum_e w_be * cost_e = <W_b, G_b>   with W_b[i,j] = sum of w over edges (i,j)

W_b is built in SBUF with gpsimd local_scatter (one [128, 1024] call per
batch PAIR; the odd batch's slots are host-offset by +512). G_b's element
f = 256*i + j lives at partition p = (f>>8)&127, slot m = (f>>15)*256 +
(f&255) — a bijection, so the host bins each edge to its home partition
(pure index/layout prep). Exact duplicate edges share a cell; their
weights are summed ON DEVICE (a reduce over a host-laid [cell, copy]
array) before the scatter.

Per batch: one vector multiply W ⊙ G (G read straight from PSUM), then a
one-hot-column matmul accumulates the product's column sums into row b of
a single [8, 512] PSUM tile (the PE has slack; this removes all per-batch
vector/scalar reductions). The [8, 512] accumulator plus per-partition
sum(w) partials go back to the host, which does the final sums, the
per-sample divide and the batch mean (the all-reduce of the sharding
hint).

Perf structure:
  - inputs arrive in a few big DMAs split across the two HWDGE queues
    (sync: scatter plan; scalar: matrices), in need-order — a single
    queue caps at ~150 GB/s
  - fp8e4m3 for P^T and D halves the big DMA and enables DoubleRow
    (2 rows/cycle) matmuls; rel err ~0.7%, well inside the 2% gate
  - all 4 W scatters are issued up front so gpsimd grinds independently
  - tensor_tensor_reduce and gpsimd-issued DMAs crash this toolchain/HW
    combo (exec-unit unrecoverable) — do not reintroduce them
  - measured dead ends: packing scatter operands into one plan tile slows
    local_scatter ~20%; moving sum(w) onto the PE via extra one-hot
    matmuls loses ~4us (the PE has no slack at its mid-ramp clock and the
    scheduler interleaves them into the G pipeline); merging dups on
    gpsimd pre-scatter is slower than the vector reduce + semaphore hop

Sharding: data-parallel over batch: 8 NeuronCores x 8 batches.
"""

from contextlib import ExitStack

import ml_dtypes
import numpy as np

import concourse.bacc as bacc
import concourse.bass as bass
import concourse.mybir as mybir
import concourse.tile as tile
from concourse.bass_utils import run_bass_kernel_spmd

B, N, E = 64, 256, 8192
NCORES = 8
BPC = B // NCORES  # batches per core
NPAIR = BPC // 2

CS = 86   # single-cell slots per partition per batch (measured max 84)
CD2 = 14  # cells with exactly 2 copies (measured max 14)
CD3 = 4   # cells with >=3 copies (measured max 3)
R3 = 10   # max copies per dup cell (measured max 10)
WDW = CD2 * 2 + CD3 * R3  # packed dup-copy words per (p, batch)
C0 = CS + CD2 + CD3

f32 = mybir.dt.float32
bf16 = mybir.dt.bfloat16
fp8 = mybir.dt.float8e4
i16 = mybir.dt.int16

NE = 512  # W/G elements per partition per batch


def _build_bass():
    nc = bacc.Bacc("TRN2", target_bir_lowering=False, debug=False)

    # ptd[p, t, kc, b2, i] = P[2t+b2, i, kc*128+p]
    ptd_in = nc.dram_tensor("ptd", [128, NPAIR, 2, 2, N], fp8, kind="ExternalInput")
    d_in = nc.dram_tensor("derr", [128, 2, N], fp8, kind="ExternalInput")
    ei_in = nc.dram_tensor("idx0", [128, BPC, C0], i16, kind="ExternalInput")
    ed_in = nc.dram_tensor("dat0", [128, BPC, C0], bf16, kind="ExternalInput")
    wd_in = nc.dram_tensor("wdup", [128, BPC, WDW], bf16, kind="ExternalInput")
    # per-(partition, batch) partials: cols [0,BPC) = sum(w*g), [BPC,2BPC) = sum(w)
    out = nc.dram_tensor("out", [128, 2 * BPC], f32, kind="ExternalOutput")
    onum = nc.dram_tensor("onum", [BPC, 2 * N], f32, kind="ExternalOutput")

    with tile.TileContext(nc) as tc, ExitStack() as ctx:
        const = ctx.enter_context(tc.tile_pool(name="const", bufs=1))
        qt_pool = ctx.enter_context(tc.tile_pool(name="qt", bufs=3))
        w_pool = ctx.enter_context(tc.tile_pool(name="w", bufs=NPAIR))
        e_pool = ctx.enter_context(tc.tile_pool(name="e", bufs=BPC))
        psA = ctx.enter_context(tc.tile_pool(name="psA", bufs=3, space="PSUM"))
        psB = ctx.enter_context(tc.tile_pool(name="psB", bufs=4, space="PSUM"))
        psC = ctx.enter_context(tc.tile_pool(name="psC", bufs=1, space="PSUM"))

        ptd_sb = const.tile([128, NPAIR, 2, 2, N], fp8)
        d_sb = const.tile([128, 2, N], fp8)
        idx_sb = const.tile([128, BPC, C0], i16)
        dat_sb = const.tile([128, BPC, C0], bf16)
        wdup_sb = const.tile([128, BPC, WDW], bf16)
        red_sb = const.tile([128, 2 * BPC], f32)
        oh_tiles = [const.tile([128, BPC], bf16, name=f"oh{b}") for b in range(BPC)]
        acc_red = const.tile([128, 1], f32)

        # ---- input DMAs: one kick per engine queue so the DIRECT2D issue
        # cost does not serialize, edge-plan tensors first in ring order
        H = BPC // 2
        nc.sync.dma_start(idx_sb[:], ei_in[:])
        nc.sync.dma_start(wdup_sb[:, 0:H], wd_in[:, 0:H])
        nc.sync.dma_start(wdup_sb[:, H:], wd_in[:, H:])
        nc.scalar.dma_start(dat_sb[:], ed_in[:])
        nc.scalar.dma_start(d_sb[:], d_in[:])
        nc.scalar.dma_start(ptd_sb[:, 0:2], ptd_in[:, 0:2])
        nc.scalar.dma_start(ptd_sb[:, 2:4], ptd_in[:, 2:4])

        nc.vector.memset(red_sb[:], 0.0)
        nc.vector.memset(acc_red[:], 1.0)  # briefly all-ones scratch
        # oh_b[p, r] = (r == b): affine iota over r with base -b
        for b in range(BPC):
            nc.gpsimd.affine_select(
                out=oh_tiles[b][:],
                in_=acc_red[:].broadcast_to((128, BPC)),
                pattern=[[-1, BPC]],
                compare_op=mybir.AluOpType.is_equal,
                fill=0.0,
                base=b,
                channel_multiplier=0,
            )

        # ---- merge dup weights straight into dat's dup tail (vector);
        # one op per batch so scatter b waits only on its own column.
        # bf16 accumulate only touches dup cells (~6% of mass): fine.
        with nc.allow_low_precision(reason="dup-cell weight sums, small mass"):
            for t in range(NPAIR):
                sl = slice(2 * t, 2 * t + 2)
                nc.vector.tensor_reduce(
                    out=dat_sb[:, sl, CS : CS + CD2],
                    in_=wdup_sb[:, sl, : CD2 * 2].rearrange(
                        "p a (c r) -> p a c r", c=CD2
                    ),
                    axis=mybir.AxisListType.X, op=mybir.AluOpType.add,
                )
                nc.vector.tensor_reduce(
                    out=dat_sb[:, sl, CS + CD2 :],
                    in_=wdup_sb[:, sl, CD2 * 2 :].rearrange(
                        "p a (c r) -> p a c r", c=CD3
                    ),
                    axis=mybir.AxisListType.X, op=mybir.AluOpType.add,
                )

        # ---- all W scatters up front (one per batch pair; odd batch slots
        # are host-offset by +512): gpsimd grinds independently
        w_pairs = []
        for t in range(NPAIR):
            w_sb = w_pool.tile([128, 2 * NE], bf16, tag="w")
            nc.gpsimd.local_scatter(
                out_ap=w_sb[:],
                data_ap=dat_sb[:, 2 * t : 2 * t + 2].rearrange("p a b -> p (a b)"),
                idxs_ap=idx_sb[:, 2 * t : 2 * t + 2].rearrange("p a b -> p (a b)"),
                channels=128,
                num_elems=2 * NE,
                num_idxs=2 * C0,
            )
            w_pairs.append(w_sb)

        # ---- main pipeline: Q one pair ahead, G per batch, fused <W,G>
        qts = [None] * NPAIR

        def do_qt(t):
            qt_sb = qt_pool.tile([128, 2, 2, N], fp8)
            for ncx in range(2):
                qps = psA.tile([128, 2, N], f32, tag="qtps")
                # fp8 DoubleRow: both k-chunks contracted in one pass
                nc.tensor.matmul(
                    qps[:].rearrange("p a b -> p (a b)"),
                    lhsT=d_sb[:, :, ncx * 128 : (ncx + 1) * 128],
                    rhs=ptd_sb[:, t].rearrange("p a b c -> p a (b c)"),
                    start=True,
                    stop=True,
                    perf_mode=mybir.MatmulPerfMode.DoubleRow,
                )
                nc.scalar.copy(qt_sb[:, ncx], qps[:])
            qts[t] = qt_sb

        acc_ps = psC.tile([BPC, 2 * N], f32, tag="acc")
        prods = []

        do_qt(0)
        for t in range(NPAIR):
            if t + 1 < NPAIR:
                do_qt(t + 1)
            for b2 in range(2):
                b = 2 * t + b2
                gps = psB.tile([128, 2, N], f32, tag="gps")
                for ic in range(2):
                    nc.tensor.matmul(
                        gps[:, ic],
                        lhsT=qts[t][:, :, b2, ic * 128 : (ic + 1) * 128],
                        rhs=ptd_sb[:, t, :, b2, :],
                        start=True,
                        stop=True,
                        perf_mode=mybir.MatmulPerfMode.DoubleRow,
                    )
                prod = e_pool.tile([128, 2 * N], bf16, tag="prod")
                nc.vector.tensor_tensor(
                    out=prod[:],
                    in0=w_pairs[t][:, b2 * NE : (b2 + 1) * NE],
                    in1=gps[:].rearrange("p a b -> p (a b)"),
                    op=mybir.AluOpType.mult,
                )
                prods.append(prod)

        # cross-partition column sums into row b of one PSUM tile:
        # out[r, m] += sum_p oh[p, b, r] * prod_b[p, m]
        for b in range(BPC):
            nc.tensor.matmul(
                acc_ps[:],
                lhsT=oh_tiles[b][:],
                rhs=prods[b][:],
                start=(b == 0),
                stop=(b == BPC - 1),
            )

        # per-batch sum(w) -> cols [BPC, 2*BPC); off the scatter critical path
        nc.vector.tensor_reduce(
            out=red_sb[:, BPC:], in_=dat_sb[:],
            axis=mybir.AxisListType.X, op=mybir.AluOpType.add,
        )
        # numerator partials: ship the [BPC, 512] accumulator; the host
        # folds the column sums into its existing all-reduce
        num_sb = const.tile([BPC, 2 * N], f32)
        nc.scalar.copy(num_sb[:], acc_ps[:])
        nc.sync.dma_start(onum[:], num_sb[:])
        nc.scalar.dma_start(out[:], red_sb[:])

    if not nc.is_finalized():
        nc.finalize()
    return nc


_NC_CACHE = {}


def _get_nc():
    if "nc" not in _NC_CACHE:
        _NC_CACHE["nc"] = _build_bass()
    return _NC_CACHE["nc"]


def _prep_in_maps(P, d_error, edge_i, edge_j, edge_w):
    P = np.asarray(P, dtype=np.float32)
    d_error = np.asarray(d_error, dtype=np.float32)
    edge_i = np.asarray(edge_i, dtype=np.int64)
    edge_j = np.asarray(edge_j, dtype=np.int64)
    edge_w = np.asarray(edge_w, dtype=np.float32)

    # ptd[p, t, kc, b2, i] = P[2t+b2, i, kc*128+p]
    PT = np.ascontiguousarray(np.transpose(P, (0, 2, 1)))  # [B, N(k), N(i)]
    PT = PT.reshape(B // 2, 2, 2, 128, N).transpose(3, 0, 2, 1, 4)
    PT = np.ascontiguousarray(PT).astype(ml_dtypes.float8_e4m3)  # [p, t, kc, b2, i]
    D = np.ascontiguousarray(d_error.reshape(2, 128, N).transpose(1, 0, 2))
    D = D.astype(ml_dtypes.float8_e4m3)

    # scatter plan: edge f = 256*i + j -> partition p = (f>>8)&127,
    # slot m = (f>>15)*256 + (f&255); odd batch of a pair offsets by +512.
    f_all = (edge_i << 8) | edge_j  # [B, E]
    p_all = (f_all >> 8) & 127
    m_all = ((f_all >> 15) << 8) | (f_all & 255)

    idx0 = np.full((B, 128, C0), -1, np.int16)
    dat0 = np.zeros((B, 128, C0), np.float32)
    wdup = np.zeros((B, 128, CD2 * 2 + CD3 * R3), np.float32)  # cast to bf16 after fill

    for b in range(B):
        pb, mb, wb = p_all[b], m_all[b], edge_w[b]
        order = np.lexsort((mb, pb))
        ps, ms, ws = pb[order], mb[order], wb[order]
        first = np.ones(E, bool)
        first[1:] = (ps[1:] != ps[:-1]) | (ms[1:] != ms[:-1])
        fidx = np.flatnonzero(first)
        ccnt = np.diff(np.append(fidx, E))
        cp, cm = ps[fidx], ms[fidx]
        for pp in np.unique(cp):
            sel = cp == pp
            cells_m = cm[sel]
            cells_n = ccnt[sel]
            cells_start = fidx[sel]
            s_slot = 0
            d2_slot = 0
            d3_slot = 0
            for cmi, cni, csi in zip(cells_m, cells_n, cells_start):
                if cni == 1:
                    assert s_slot < CS
                    idx0[b, pp, s_slot] = cmi
                    dat0[b, pp, s_slot] = ws[csi]
                    s_slot += 1
                elif cni == 2:
                    assert d2_slot < CD2
                    idx0[b, pp, CS + d2_slot] = cmi
                    wdup[b, pp, 2 * d2_slot : 2 * d2_slot + 2] = ws[csi : csi + 2]
                    d2_slot += 1
                else:
                    assert d3_slot < CD3 and cni <= R3
                    idx0[b, pp, CS + CD2 + d3_slot] = cmi
                    base = CD2 * 2 + R3 * d3_slot
                    wdup[b, pp, base : base + cni] = ws[csi : csi + cni]
                    d3_slot += 1
    for b in range(1, B, 2):  # pair-scatter: odd batch lives at slots [512, 1024)
        valid = idx0[b] >= 0
        idx0[b][valid] += NE

    dat0 = dat0.astype(ml_dtypes.bfloat16)
    wdup = wdup.astype(ml_dtypes.bfloat16)
    # -> [p, b, ...]
    idx0 = np.ascontiguousarray(idx0.transpose(1, 0, 2))
    dat0 = np.ascontiguousarray(dat0.transpose(1, 0, 2))
    wdup = np.ascontiguousarray(wdup.transpose(1, 0, 2))

    in_maps = []
    for c in range(NCORES):
        sl = slice(c * BPC, (c + 1) * BPC)
        in_maps.append(
            {
                "ptd": np.ascontiguousarray(PT[:, c * NPAIR : (c + 1) * NPAIR]),
                "derr": D,
                "idx0": np.ascontiguousarray(idx0[:, sl]),
                "dat0": np.ascontiguousarray(dat0[:, sl]),
                "wdup": np.ascontiguousarray(wdup[:, sl]),
            }
        )
    return in_maps


def run(P, d_error, edge_i, edge_j, edge_w, trace=False):
    """Run on 8 cores; returns (loss_scalar, BassKernelResults)."""
    nc = _get_nc()
    in_maps = _prep_in_maps(P, d_error, edge_i, edge_j, edge_w)
    res = run_bass_kernel_spmd(
        nc, in_maps, core_ids=list(range(NCORES)), trace=trace
    )
    # host-side all-reduce: loss = mean_b( sl_b / max(sw_b, 1e-8) )
    acc = 0.0
    for r in res.results:
        part = np.asarray(r["out"], dtype=np.float64).reshape(128, 2 * BPC)
        sl = np.asarray(r["onum"], dtype=np.float64).reshape(BPC, 2 * N).sum(axis=1)
        sw = part[:, BPC:].sum(axis=0)
        acc += float(np.sum(sl / np.maximum(sw, 1e-8)))
    loss = np.float32(acc / B)
    return loss, res


def kernel(P, d_error, edge_i, edge_j, edge_w):
    loss, _ = run(P, d_error, edge_i, edge_j, edge_w, trace=False)
    return np.asarray(loss, dtype=np.float32)
